# revision 32
# baseline (speedup 1.0000x reference)
"""Causal self-attention (B=2, T=2048, C=1024, H=16, D=64) on 8 TRN2 NeuronCores.

Sharding: core c handles batch b = c//4 and 4 heads [4*(c%4), 4*(c%4)+4)
(tensor-parallel over heads x data-parallel over batch). Each core:
  - qT/kT = W.T @ x.T (transposed layouts, contraction over C on partitions)
  - v in natural [s, j] layout, augmented per head with 64 columns of ones
    so each AV matmul emits both y rows (0:64) and replicated softmax
    denominators (64:128) in one PSUM bank
  - causal flash-style attention per head pair (row-packed K=64 QK^T
    matmuls, exp on ScalarE with fused 1/sqrt(D) scale, no max-subtraction
    -- logits are O(6) for this problem family)
  - partial output projection over its 256 head-channels
Host sums the 4 partial projections per batch and adds bp.

All matmuls run in float32r (TF32-like, ~1 cyc/row, rel err ~1.5e-4).
"""

import numpy as np

import concourse.bass as bass
import concourse.mybir as mybir
import concourse.tile as tile
from concourse import bacc
from concourse.bass import ts
from concourse.bass_utils import run_bass_kernel_spmd

P = 128
B, T, C, H, D = 2, 2048, 1024, 16, 64
N_CORES = 8
HPC = 4  # heads per core
JPC = HPC * D  # 256 head-channels per core
KO = C // P  # 8 contraction subtiles
F32 = mybir.dt.float32
F32R = mybir.dt.float32r
BF16 = mybir.dt.bfloat16
AF = mybir.ActivationFunctionType
MUL = mybir.AluOpType.mult
ADD = mybir.AluOpType.add


def _build(T_=T):
    """Build + compile the per-core Bass kernel for sequence length T_."""
    TBs = T_ // 512  # number of 512-wide t blocks
    NSO = T_ // 128  # number of 128-row s tiles
    nc = bacc.Bacc(None, target_bir_lowering=False)

    xT4 = nc.dram_tensor("xT4", [TBs, P, KO, 512], F32R, kind="ExternalInput")
    wq = nc.dram_tensor("wq", [P, KO, JPC], F32R, kind="ExternalInput")
    wk = nc.dram_tensor("wk", [P, KO, JPC], F32R, kind="ExternalInput")
    wv = nc.dram_tensor("wv", [P, KO, JPC], F32R, kind="ExternalInput")
    wp = nc.dram_tensor("wp", [P, 2, C], F32R, kind="ExternalInput")
    bq = nc.dram_tensor("bq", [P, 2], F32, kind="ExternalInput")
    bk = nc.dram_tensor("bk", [P, 2], F32, kind="ExternalInput")
    bv = nc.dram_tensor("bv", [JPC], F32, kind="ExternalInput")
    masks = nc.dram_tensor("masks", [P, P], F32, kind="ExternalInput")
    out = nc.dram_tensor("out", [T_, C], F32, kind="ExternalOutput")

    with tile.TileContext(nc) as tc:
        with (
            tc.tile_pool(name="consts", bufs=1) as consts,
            tc.tile_pool(name="resid", bufs=1) as resid,
            tc.tile_pool(name="xq_pool", bufs=2) as xq_pool,
            tc.tile_pool(name="pt_pool", bufs=3) as pt_pool,
            tc.tile_pool(name="work", bufs=3) as work,
            tc.tile_pool(name="psum", bufs=1, space="PSUM") as psum,
        ):
            # ---- constants (ordered so first-needed data DMAs first) ----
            wq_sb = consts.tile([P, KO, JPC], F32R, name="wq_sb")
            xq0 = xq_pool.tile([P, KO, 512], F32R, tag="xq", name="xq")
            # per-ko interleaved chunks: the ko=0 matmuls can start after
            # ~1/8th of the data has landed
            for ko in range(KO):
                nc.sync.dma_start(wq_sb[:, ko, :], wq[:, ko, :])
                nc.sync.dma_start(xq0[:, ko, :], xT4[0, :, ko, :])
            wk_sb = consts.tile([P, KO, JPC], F32R, name="wk_sb")
            nc.sync.dma_start(wk_sb[:], wk[:])
            wv_sb = consts.tile([P, KO, JPC], F32R, name="wv_sb")
            nc.sync.dma_start(wv_sb[:], wv[:])
            bqc = consts.tile([P, 2], F32, name="bqc")
            nc.sync.dma_start(bqc[:], bq[:])
            bkc = consts.tile([P, 2], F32, name="bkc")
            nc.sync.dma_start(bkc[:], bk[:])
            bv_bc = consts.tile([P, JPC], F32, name="bv_bc")
            bv_ap = bv[:]
            nc.sync.dma_start(
                bv_bc[:],
                bass.AP(tensor=bv_ap.tensor, offset=0, ap=[[0, P], [1, JPC]]),
            )
            masks_sb = consts.tile([P, P], F32, name="masks_sb")
            nc.sync.dma_start(masks_sb[:], masks[:])
            wp_sb = consts.tile([P, 2, C], F32R, name="wp_sb")
            nc.sync.dma_start(wp_sb[:], wp[:])
            ones_f32 = consts.tile([P, D], F32, name="ones_f32")
            nc.vector.memset(ones_f32[:], 1.0)

            # ---- residents ----
            qT = resid.tile([P, 2, T_], F32R, name="qT")
            kT = resid.tile([P, 2, T_], F32R, name="kT")
            # v: [s-partition, s-tile, head-major columns of [v_h | ones]]
            v_sb = resid.tile([P, NSO, HPC * P], F32R, name="v_sb")
            yheadsT = resid.tile([P, 2, T_], F32R, name="yheadsT")

            # ones columns of v (broadcast one [P, D] tile over s-tiles/heads)
            nc.vector.tensor_copy(
                v_sb.rearrange("p so (h c) -> p so h c", c=P)[:, :, :, D:],
                ones_f32[:, None, None, :].broadcast_to([P, NSO, HPC, D]),
            )

            # HAM warm-up: ~7us of dummy matmuls while the first input DMAs
            # stream, so real work starts with the PE clock at 2.4 GHz
            warm_src = consts.tile([P, 512], F32R, name="warm_src")
            nc.vector.tensor_copy(
                warm_src[:], ones_f32[:, None, :].broadcast_to([P, 8, D])
            )
            for wi in range(16):
                wps = psum.tile([P, 2, 512], F32, tag="st", bufs=2, name="wps")
                nc.tensor.matmul(
                    wps[:, 0, :],
                    warm_src[:, 0:P],
                    warm_src[:],
                    start=True,
                    stop=True,
                )

            # ---- QKV projections for one 512-column quarter of x ----
            def qkv_quarter(qtr, xq):
                for u in qkv_units(qtr, xq):
                    u()

            def qkv_units(qtr, xq):
                """Deferred emission units (~1.7-3.5us of PE work each)."""

                def qk_unit(w_sb, bias_col, dstT):
                    def emit():
                        ps = psum.tile(
                            [P, 2, 512], F32, tag="yt", bufs=2, name="ps_qk"
                        )
                        for jo in range(2):
                            for ko in range(KO):
                                nc.tensor.matmul(
                                    ps[:, jo, :],
                                    w_sb[:, ko, ts(jo, P)],
                                    xq[:, ko, :],
                                    start=(ko == 0),
                                    stop=(ko == KO - 1),
                                )
                            nc.vector.tensor_scalar_add(
                                dstT[:, jo, ts(qtr, 512)],
                                ps[:, jo, :],
                                bias_col[:, jo : jo + 1],
                            )

                    return emit

                def v_unit(tp):
                    def emit():
                        ps = psum.tile(
                            [P, 2, 512], F32, tag="yt", bufs=2, name="ps_v"
                        )
                        for sub in range(2):
                            tt = 2 * tp + sub
                            so = qtr * 4 + tt
                            for ko in range(KO):
                                nc.tensor.matmul(
                                    ps[:, sub, :JPC],
                                    xq[:, ko, ts(tt, P)],
                                    wv_sb[:, ko, :],
                                    start=(ko == 0),
                                    stop=(ko == KO - 1),
                                )
                            for h in range(HPC):
                                nc.vector.tensor_tensor(
                                    v_sb[:, so, h * P : h * P + D],
                                    ps[:, sub, ts(h, D)],
                                    bv_bc[:, ts(h, D)],
                                    ADD,
                                )

                    return emit

                return [
                    qk_unit(wq_sb, bqc, qT),
                    qk_unit(wk_sb, bkc, kT),
                    v_unit(0),
                    v_unit(1),
                ]

            # ---- attention for head pair jo, one 512-row t block ----
            # `fill`: deferred work units interleaved between regions so the
            # PE stays fed while the region chain paces on ScalarE's exp
            def attend_tb(jo, tb, fill=()):
                yps = psum.tile([P, 2, 512], F32, tag="yt", bufs=2, name="yps")
                # diagonal s-tiles first (m=0 full tile starts the psum
                # accumulation), then the full off-diagonal tiles
                order = [(4 * tb + m, m) for m in (0, 3, 2, 1) if 4 * tb + m < 4 * (tb + 1)]
                order += [(si, None) for si in range(4 * tb)]
                n_mm = len(order)

                def emit_st(si, m):
                    tw0 = 0 if m is None else P * m
                    stp = psum.tile(
                        [P, 2, 512], F32, tag="st", bufs=2, name="stp"
                    )
                    for hh in range(2):
                        sl = slice(64 * hh, 64 * hh + 64)
                        nc.tensor.matmul(
                            stp[:, hh, tw0:],
                            kT[sl, jo, ts(si, P)],
                            qT[sl, jo, tb * 512 + tw0 : (tb + 1) * 512],
                            start=True,
                            stop=True,
                            tile_position=(64 * hh, 0),
                        )
                    pt = pt_pool.tile([P, 2, 512], F32R, tag="pt", name="pt")
                    nc.scalar.activation(
                        pt[:, :, tw0:],
                        stp[:, :, tw0:],
                        AF.Exp,
                        scale=float(1.0 / np.sqrt(D)),
                    )
                    if m is not None:
                        # triangle mask on the leading 128 columns
                        nc.vector.tensor_tensor(
                            pt[:, :, tw0 : tw0 + P],
                            pt[:, :, tw0 : tw0 + P],
                            masks_sb[:, None, :].broadcast_to([P, 2, P]),
                            MUL,
                        )
                    return pt, tw0

                def emit_av(si, pt, tw0, idx):
                    for hh in range(2):
                        h = 2 * jo + hh
                        nc.tensor.matmul(
                            yps[:, hh, tw0:],
                            v_sb[:, si, ts(h, P)],
                            pt[:, hh, tw0:],
                            start=(idx == 0),
                            stop=(idx == n_mm - 1),
                        )

                # software-pipelined: keep TWO ST/exp regions in flight ahead
                # of each AV pair so the exp + diagonal-mask latency never
                # stalls the PE (pt_pool bufs must be >= depth + 1)
                fill = list(fill)
                pending = []
                for idx, (si, m) in enumerate(order):
                    pt, tw0 = emit_st(si, m)
                    pending.append((si, pt, tw0, idx))
                    if len(pending) > 2:
                        emit_av(*pending.pop(0))
                    if fill and idx % 2 == 1:
                        fill.pop(0)()
                for p in pending:
                    emit_av(*p)
                for u in fill:
                    u()

                # 1/s = exp(-ln(s)) on ScalarE, both heads in one op: Ln shares
                # the exp table set; sums are 64-row replicated in PSUM
                ls = work.tile([64, 2, 512], F32, tag="ls", name="ls")
                nc.scalar.activation(ls[:], yps[64:128, :, :], AF.Ln)
                rc = work.tile([64, 2, 512], F32, tag="rc", name="rc")
                nc.scalar.activation(rc[:], ls[:], AF.Exp, scale=-1.0)
                for hh in range(2):
                    nc.vector.tensor_tensor(
                        yheadsT[64 * hh : 64 * hh + 64, jo, ts(tb, 512)],
                        yps[0:64, hh, :],
                        rc[:, hh, :],
                        MUL,
                    )

            def proj_unit(tt):
                def emit():
                    ps = psum.tile(
                        [P, 2, 512], F32, tag="yt", bufs=2, name="ps_pr"
                    )
                    for ob in range(2):
                        for jo in range(2):
                            nc.tensor.matmul(
                                ps[:, ob, :],
                                yheadsT[:, jo, ts(tt, P)],
                                wp_sb[:, jo, ts(ob, 512)],
                                start=(jo == 0),
                                stop=(jo == 1),
                            )
                    o = work.tile([P, 2, 512], F32, tag="po", name="po")
                    nc.vector.tensor_copy(o[:], ps[:])
                    nc.sync.dma_start(
                        out[ts(tt, P), :], o.rearrange("p a b -> p (a b)")
                    )

                return emit

            def proj_units(tb):
                return [proj_unit(tt) for tt in range(4 * tb, 4 * tb + 4)]

            for tb in range(TBs):
                # prefetch next quarter of x a full phase early so the
                # qkv fill units never wait on the DMA
                xq_n = None
                if tb + 1 < TBs:
                    xq_n = xq_pool.tile([P, KO, 512], F32R, tag="xq", name="xq")
                    nc.sync.dma_start(xq_n[:], xT4[tb + 1])
                if tb == 0:
                    with nc.named_scope("qkv"):
                        qkv_quarter(0, xq0)
                    with nc.named_scope("attn0"):
                        attend_tb(0, 0)
                else:
                    # qkv for this quarter is interleaved into the previous
                    # iteration's attention; here interleave nothing extra
                    with nc.named_scope("attn0"):
                        attend_tb(0, tb)
                # interleave next quarter's qkv into attn1, plus the
                # previous t-block's projection
                fill = []
                if xq_n is not None:
                    fill += qkv_units(tb + 1, xq_n)
                if tb > 0:
                    fill += proj_units(tb - 1)
                with nc.named_scope("attn1"):
                    attend_tb(1, tb, fill)
            with nc.named_scope("proj"):
                for u in proj_units(TBs - 1):
                    u()

    nc.compile()
    _fixup_act_table_loads(nc)
    return nc


def _fixup_act_table_loads(nc):
    """All activations here need only {Exp, Ln}, both present in the
    natural_log_exp_and_others set — but the table-load pass picks the
    first set per function and ping-pongs (1.3us per reload). Point the
    first load at the combined set and drop the rest."""
    from concourse.hw_specs import get_activation_tables

    tables = get_activation_tables(nc.m.arch)
    names = list(tables)
    combined = names.index("natural_log_exp_and_others")
    used = {AF.Exp, AF.Ln}
    assert used <= tables["natural_log_exp_and_others"]
    first = True
    for b in nc.main_func.blocks:
        keep = []
        for inst in b.instructions:
            if type(inst).__name__ == "InstLoadActFuncSet":
                assert inst.sync_info is None
                if first:
                    inst.act_func_set_id = combined
                    keep.append(inst)
                    first = False
                continue
            keep.append(inst)
        b.instructions[:] = keep


_CACHE = {}


def _get_nc(T_=T):
    if T_ not in _CACHE:
        _CACHE[T_] = _build(T_)
    return _CACHE[T_]


def _make_masks():
    """mask[s_local, t_local] = 1.0 where t_local >= s_local (incl. diag)."""
    t_idx = np.arange(P)[None, :]
    s_idx = np.arange(P)[:, None]
    return (t_idx >= s_idx).astype(np.float32)


def _prep_w(W_cols):
    """[C, JPC] -> [P, KO, JPC] with c = ko*128 + p."""
    return np.ascontiguousarray(W_cols.reshape(KO, P, JPC).transpose(1, 0, 2))


def _prep_core_inputs(xb, Wq_s, bq_s, Wk_s, bk_s, Wv_s, bv_s, Wp_s, T_=T):
    xT = xb.T  # [C, T_]
    xT4 = np.ascontiguousarray(
        xT.reshape(KO, P, T_ // 512, 512).transpose(2, 1, 0, 3)
    )
    return {
        "xT4": xT4,
        "wq": _prep_w(Wq_s),
        "wk": _prep_w(Wk_s),
        "wv": _prep_w(Wv_s),
        "wp": np.ascontiguousarray(Wp_s.reshape(2, P, C).transpose(1, 0, 2)),
        "bq": np.ascontiguousarray(bq_s.reshape(2, P).T),
        "bk": np.ascontiguousarray(bk_s.reshape(2, P).T),
        "bv": np.ascontiguousarray(bv_s),
        "masks": _make_masks(),
    }


def _shard_inputs(x, Wq, bq, Wk, bk, Wv, bv, Wp):
    in_maps = []
    for c in range(N_CORES):
        b = c // 4
        g = c % 4
        js = slice(g * JPC, (g + 1) * JPC)
        in_maps.append(
            _prep_core_inputs(
                x[b], Wq[:, js], bq[js], Wk[:, js], bk[js],
                Wv[:, js], bv[js], Wp[js, :],
            )
        )
    return in_maps


def _combine(results, bp):
    out = np.empty((B, T, C), dtype=np.float32)
    for b in range(B):
        acc = results[4 * b]["out"].astype(np.float32).copy()
        for g in range(1, 4):
            acc += results[4 * b + g]["out"]
        out[b] = acc + bp[None, :]
    return out


def _run(inputs, trace=False, **kwargs):
    nc = _get_nc(T)
    in_maps = _shard_inputs(
        np.asarray(inputs["x"], dtype=np.float32),
        np.asarray(inputs["Wq"], dtype=np.float32),
        np.asarray(inputs["bq"], dtype=np.float32),
        np.asarray(inputs["Wk"], dtype=np.float32),
        np.asarray(inputs["bk"], dtype=np.float32),
        np.asarray(inputs["Wv"], dtype=np.float32),
        np.asarray(inputs["bv"], dtype=np.float32),
        np.asarray(inputs["Wp"], dtype=np.float32),
    )
    res = run_bass_kernel_spmd(
        nc, in_maps, core_ids=list(range(N_CORES)), trace=trace, **kwargs
    )
    full = _combine(res.results, np.asarray(inputs["bp"], dtype=np.float32))
    return full, res


def kernel(**inputs) -> np.ndarray:
    full, _ = _run(inputs, trace=False)
    return full


# revision 33
# speedup vs baseline: 1.0090x; 1.0090x over previous
"""Causal self-attention (B=2, T=2048, C=1024, H=16, D=64) on 8 TRN2 NeuronCores.

Sharding: core c handles batch b = c//4 and 4 heads [4*(c%4), 4*(c%4)+4)
(tensor-parallel over heads x data-parallel over batch). Each core:
  - qT/kT = W.T @ x.T (transposed layouts, contraction over C on partitions)
  - v in natural [s, j] layout, augmented per head with 64 columns of ones
    so each AV matmul emits both y rows (0:64) and replicated softmax
    denominators (64:128) in one PSUM bank
  - causal flash-style attention per head pair (row-packed K=64 QK^T
    matmuls, exp on ScalarE with fused 1/sqrt(D) scale, no max-subtraction
    -- logits are O(6) for this problem family)
  - partial output projection over its 256 head-channels
Host sums the 4 partial projections per batch and adds bp.

All matmuls run in float32r (TF32-like, ~1 cyc/row, rel err ~1.5e-4).
"""

import numpy as np

import concourse.bass as bass
import concourse.mybir as mybir
import concourse.tile as tile
from concourse import bacc
from concourse.bass import ts
from concourse.bass_utils import run_bass_kernel_spmd

P = 128
B, T, C, H, D = 2, 2048, 1024, 16, 64
N_CORES = 8
HPC = 4  # heads per core
JPC = HPC * D  # 256 head-channels per core
KO = C // P  # 8 contraction subtiles
F32 = mybir.dt.float32
F32R = mybir.dt.float32r
BF16 = mybir.dt.bfloat16
AF = mybir.ActivationFunctionType
MUL = mybir.AluOpType.mult
ADD = mybir.AluOpType.add


def _build(T_=T):
    """Build + compile the per-core Bass kernel for sequence length T_."""
    TBs = T_ // 512  # number of 512-wide t blocks
    NSO = T_ // 128  # number of 128-row s tiles
    nc = bacc.Bacc(None, target_bir_lowering=False)

    xT4 = nc.dram_tensor("xT4", [TBs, P, KO, 512], F32R, kind="ExternalInput")
    wq = nc.dram_tensor("wq", [P, KO, JPC], F32R, kind="ExternalInput")
    wk = nc.dram_tensor("wk", [P, KO, JPC], F32R, kind="ExternalInput")
    wv = nc.dram_tensor("wv", [P, KO, JPC], F32R, kind="ExternalInput")
    wp = nc.dram_tensor("wp", [P, 2, C], F32R, kind="ExternalInput")
    bq = nc.dram_tensor("bq", [P, 2], F32, kind="ExternalInput")
    bk = nc.dram_tensor("bk", [P, 2], F32, kind="ExternalInput")
    bv = nc.dram_tensor("bv", [JPC], F32, kind="ExternalInput")
    masks = nc.dram_tensor("masks", [P, P], F32, kind="ExternalInput")
    out = nc.dram_tensor("out", [T_, C], F32, kind="ExternalOutput")

    with tile.TileContext(nc) as tc:
        with (
            tc.tile_pool(name="consts", bufs=1) as consts,
            tc.tile_pool(name="resid", bufs=1) as resid,
            tc.tile_pool(name="xq_pool", bufs=2) as xq_pool,
            tc.tile_pool(name="pt_pool", bufs=3) as pt_pool,
            tc.tile_pool(name="work", bufs=3) as work,
            tc.tile_pool(name="psum", bufs=1, space="PSUM") as psum,
        ):
            # ---- constants (ordered so first-needed data DMAs first) ----
            wq_sb = consts.tile([P, KO, JPC], F32R, name="wq_sb")
            nc.sync.dma_start(wq_sb[:], wq[:])
            xq0 = xq_pool.tile([P, KO, 512], F32R, tag="xq", name="xq")
            nc.sync.dma_start(xq0[:], xT4[0])
            wk_sb = consts.tile([P, KO, JPC], F32R, name="wk_sb")
            nc.sync.dma_start(wk_sb[:], wk[:])
            wv_sb = consts.tile([P, KO, JPC], F32R, name="wv_sb")
            nc.sync.dma_start(wv_sb[:], wv[:])
            bqc = consts.tile([P, 2], F32, name="bqc")
            nc.sync.dma_start(bqc[:], bq[:])
            bkc = consts.tile([P, 2], F32, name="bkc")
            nc.sync.dma_start(bkc[:], bk[:])
            bv_bc = consts.tile([P, JPC], F32, name="bv_bc")
            bv_ap = bv[:]
            nc.sync.dma_start(
                bv_bc[:],
                bass.AP(tensor=bv_ap.tensor, offset=0, ap=[[0, P], [1, JPC]]),
            )
            masks_sb = consts.tile([P, P], F32, name="masks_sb")
            nc.sync.dma_start(masks_sb[:], masks[:])
            wp_sb = consts.tile([P, 2, C], F32R, name="wp_sb")
            nc.sync.dma_start(wp_sb[:], wp[:])
            ones_f32 = consts.tile([P, D], F32, name="ones_f32")
            nc.vector.memset(ones_f32[:], 1.0)

            # ---- residents ----
            qT = resid.tile([P, 2, T_], F32R, name="qT")
            kT = resid.tile([P, 2, T_], F32R, name="kT")
            # v: [s-partition, s-tile, head-major columns of [v_h | ones]]
            v_sb = resid.tile([P, NSO, HPC * P], F32R, name="v_sb")
            yheadsT = resid.tile([P, 2, T_], F32R, name="yheadsT")

            # ones columns of v (broadcast one [P, D] tile over s-tiles/heads)
            nc.vector.tensor_copy(
                v_sb.rearrange("p so (h c) -> p so h c", c=P)[:, :, :, D:],
                ones_f32[:, None, None, :].broadcast_to([P, NSO, HPC, D]),
            )

            # HAM warm-up: ~7us of dummy matmuls while the first input DMAs
            # stream, so real work starts with the PE clock at 2.4 GHz
            warm_src = consts.tile([P, 512], F32R, name="warm_src")
            nc.vector.tensor_copy(
                warm_src[:], ones_f32[:, None, :].broadcast_to([P, 8, D])
            )
            for wi in range(16):
                wps = psum.tile([P, 2, 512], F32, tag="st", bufs=2, name="wps")
                nc.tensor.matmul(
                    wps[:, 0, :],
                    warm_src[:, 0:P],
                    warm_src[:],
                    start=True,
                    stop=True,
                )

            # ---- QKV projections for one 512-column quarter of x ----
            def qkv_quarter(qtr, xq):
                for u in qkv_units(qtr, xq):
                    u()

            def qkv_units(qtr, xq):
                """Deferred emission units (~1.7-3.5us of PE work each)."""

                def qk_unit(w_sb, bias_col, dstT):
                    def emit():
                        ps = psum.tile(
                            [P, 2, 512], F32, tag="yt", bufs=2, name="ps_qk"
                        )
                        for jo in range(2):
                            for ko in range(KO):
                                nc.tensor.matmul(
                                    ps[:, jo, :],
                                    w_sb[:, ko, ts(jo, P)],
                                    xq[:, ko, :],
                                    start=(ko == 0),
                                    stop=(ko == KO - 1),
                                )
                            nc.vector.tensor_scalar_add(
                                dstT[:, jo, ts(qtr, 512)],
                                ps[:, jo, :],
                                bias_col[:, jo : jo + 1],
                            )

                    return emit

                def v_unit(tp):
                    def emit():
                        ps = psum.tile(
                            [P, 2, 512], F32, tag="yt", bufs=2, name="ps_v"
                        )
                        for sub in range(2):
                            tt = 2 * tp + sub
                            so = qtr * 4 + tt
                            for ko in range(KO):
                                nc.tensor.matmul(
                                    ps[:, sub, :JPC],
                                    xq[:, ko, ts(tt, P)],
                                    wv_sb[:, ko, :],
                                    start=(ko == 0),
                                    stop=(ko == KO - 1),
                                )
                            for h in range(HPC):
                                nc.vector.tensor_tensor(
                                    v_sb[:, so, h * P : h * P + D],
                                    ps[:, sub, ts(h, D)],
                                    bv_bc[:, ts(h, D)],
                                    ADD,
                                )

                    return emit

                return [
                    qk_unit(wq_sb, bqc, qT),
                    qk_unit(wk_sb, bkc, kT),
                    v_unit(0),
                    v_unit(1),
                ]

            # ---- attention for head pair jo, one 512-row t block ----
            # `fill`: deferred work units interleaved between regions so the
            # PE stays fed while the region chain paces on ScalarE's exp
            def attend_tb(jo, tb, fill=()):
                yps = psum.tile([P, 2, 512], F32, tag="yt", bufs=2, name="yps")
                # diagonal s-tiles first (m=0 full tile starts the psum
                # accumulation), then the full off-diagonal tiles
                order = [(4 * tb + m, m) for m in (0, 3, 2, 1) if 4 * tb + m < 4 * (tb + 1)]
                order += [(si, None) for si in range(4 * tb)]
                n_mm = len(order)

                def emit_st(si, m):
                    tw0 = 0 if m is None else P * m
                    stp = psum.tile(
                        [P, 2, 512], F32, tag="st", bufs=2, name="stp"
                    )
                    for hh in range(2):
                        sl = slice(64 * hh, 64 * hh + 64)
                        nc.tensor.matmul(
                            stp[:, hh, tw0:],
                            kT[sl, jo, ts(si, P)],
                            qT[sl, jo, tb * 512 + tw0 : (tb + 1) * 512],
                            start=True,
                            stop=True,
                            tile_position=(64 * hh, 0),
                        )
                    pt = pt_pool.tile([P, 2, 512], F32R, tag="pt", name="pt")
                    nc.scalar.activation(
                        pt[:, :, tw0:],
                        stp[:, :, tw0:],
                        AF.Exp,
                        scale=float(1.0 / np.sqrt(D)),
                    )
                    if m is not None:
                        # triangle mask on the leading 128 columns
                        nc.vector.tensor_tensor(
                            pt[:, :, tw0 : tw0 + P],
                            pt[:, :, tw0 : tw0 + P],
                            masks_sb[:, None, :].broadcast_to([P, 2, P]),
                            MUL,
                        )
                    return pt, tw0

                def emit_av(si, pt, tw0, idx):
                    for hh in range(2):
                        h = 2 * jo + hh
                        nc.tensor.matmul(
                            yps[:, hh, tw0:],
                            v_sb[:, si, ts(h, P)],
                            pt[:, hh, tw0:],
                            start=(idx == 0),
                            stop=(idx == n_mm - 1),
                        )

                # software-pipelined: keep TWO ST/exp regions in flight ahead
                # of each AV pair so the exp + diagonal-mask latency never
                # stalls the PE (pt_pool bufs must be >= depth + 1)
                fill = list(fill)
                pending = []
                for idx, (si, m) in enumerate(order):
                    pt, tw0 = emit_st(si, m)
                    pending.append((si, pt, tw0, idx))
                    if len(pending) > 2:
                        emit_av(*pending.pop(0))
                    if fill and idx % 2 == 1:
                        fill.pop(0)()
                for p in pending:
                    emit_av(*p)
                for u in fill:
                    u()

                # 1/s = exp(-ln(s)) on ScalarE, both heads in one op: Ln shares
                # the exp table set; sums are 64-row replicated in PSUM
                ls = work.tile([64, 2, 512], F32, tag="ls", name="ls")
                nc.scalar.activation(ls[:], yps[64:128, :, :], AF.Ln)
                rc = work.tile([64, 2, 512], F32, tag="rc", name="rc")
                nc.scalar.activation(rc[:], ls[:], AF.Exp, scale=-1.0)
                for hh in range(2):
                    nc.vector.tensor_tensor(
                        yheadsT[64 * hh : 64 * hh + 64, jo, ts(tb, 512)],
                        yps[0:64, hh, :],
                        rc[:, hh, :],
                        MUL,
                    )

            def proj_unit(tt):
                def emit():
                    ps = psum.tile(
                        [P, 2, 512], F32, tag="yt", bufs=2, name="ps_pr"
                    )
                    for ob in range(2):
                        for jo in range(2):
                            nc.tensor.matmul(
                                ps[:, ob, :],
                                yheadsT[:, jo, ts(tt, P)],
                                wp_sb[:, jo, ts(ob, 512)],
                                start=(jo == 0),
                                stop=(jo == 1),
                            )
                    o = work.tile([P, 2, 512], F32, tag="po", name="po")
                    nc.vector.tensor_copy(o[:], ps[:])
                    nc.sync.dma_start(
                        out[ts(tt, P), :], o.rearrange("p a b -> p (a b)")
                    )

                return emit

            def proj_units(tb):
                return [proj_unit(tt) for tt in range(4 * tb, 4 * tb + 4)]

            for tb in range(TBs):
                # prefetch next quarter of x a full phase early so the
                # qkv fill units never wait on the DMA
                xq_n = None
                if tb + 1 < TBs:
                    xq_n = xq_pool.tile([P, KO, 512], F32R, tag="xq", name="xq")
                    nc.sync.dma_start(xq_n[:], xT4[tb + 1])
                if tb == 0:
                    with nc.named_scope("qkv"):
                        qkv_quarter(0, xq0)
                    with nc.named_scope("attn0"):
                        attend_tb(0, 0)
                else:
                    # qkv for this quarter is interleaved into the previous
                    # iteration's attention; here interleave nothing extra
                    with nc.named_scope("attn0"):
                        attend_tb(0, tb)
                # interleave next quarter's qkv into attn1, plus the
                # previous t-block's projection
                fill = []
                if xq_n is not None:
                    fill += qkv_units(tb + 1, xq_n)
                if tb > 0:
                    fill += proj_units(tb - 1)
                with nc.named_scope("attn1"):
                    attend_tb(1, tb, fill)
            with nc.named_scope("proj"):
                for u in proj_units(TBs - 1):
                    u()

    nc.compile()
    _fixup_act_table_loads(nc)
    return nc


def _fixup_act_table_loads(nc):
    """All activations here need only {Exp, Ln}, both present in the
    natural_log_exp_and_others set — but the table-load pass picks the
    first set per function and ping-pongs (1.3us per reload). Point the
    first load at the combined set and drop the rest."""
    from concourse.hw_specs import get_activation_tables

    tables = get_activation_tables(nc.m.arch)
    names = list(tables)
    combined = names.index("natural_log_exp_and_others")
    used = {AF.Exp, AF.Ln}
    assert used <= tables["natural_log_exp_and_others"]
    first = True
    for b in nc.main_func.blocks:
        keep = []
        for inst in b.instructions:
            if type(inst).__name__ == "InstLoadActFuncSet":
                assert inst.sync_info is None
                if first:
                    inst.act_func_set_id = combined
                    keep.append(inst)
                    first = False
                continue
            keep.append(inst)
        b.instructions[:] = keep


_CACHE = {}


def _get_nc(T_=T):
    if T_ not in _CACHE:
        _CACHE[T_] = _build(T_)
    return _CACHE[T_]


def _make_masks():
    """mask[s_local, t_local] = 1.0 where t_local >= s_local (incl. diag)."""
    t_idx = np.arange(P)[None, :]
    s_idx = np.arange(P)[:, None]
    return (t_idx >= s_idx).astype(np.float32)


def _prep_w(W_cols):
    """[C, JPC] -> [P, KO, JPC] with c = ko*128 + p."""
    return np.ascontiguousarray(W_cols.reshape(KO, P, JPC).transpose(1, 0, 2))


def _prep_core_inputs(xb, Wq_s, bq_s, Wk_s, bk_s, Wv_s, bv_s, Wp_s, T_=T):
    xT = xb.T  # [C, T_]
    xT4 = np.ascontiguousarray(
        xT.reshape(KO, P, T_ // 512, 512).transpose(2, 1, 0, 3)
    )
    return {
        "xT4": xT4,
        "wq": _prep_w(Wq_s),
        "wk": _prep_w(Wk_s),
        "wv": _prep_w(Wv_s),
        "wp": np.ascontiguousarray(Wp_s.reshape(2, P, C).transpose(1, 0, 2)),
        "bq": np.ascontiguousarray(bq_s.reshape(2, P).T),
        "bk": np.ascontiguousarray(bk_s.reshape(2, P).T),
        "bv": np.ascontiguousarray(bv_s),
        "masks": _make_masks(),
    }


def _shard_inputs(x, Wq, bq, Wk, bk, Wv, bv, Wp):
    in_maps = []
    for c in range(N_CORES):
        b = c // 4
        g = c % 4
        js = slice(g * JPC, (g + 1) * JPC)
        in_maps.append(
            _prep_core_inputs(
                x[b], Wq[:, js], bq[js], Wk[:, js], bk[js],
                Wv[:, js], bv[js], Wp[js, :],
            )
        )
    return in_maps


def _combine(results, bp):
    out = np.empty((B, T, C), dtype=np.float32)
    for b in range(B):
        acc = results[4 * b]["out"].astype(np.float32).copy()
        for g in range(1, 4):
            acc += results[4 * b + g]["out"]
        out[b] = acc + bp[None, :]
    return out


def _run(inputs, trace=False, **kwargs):
    nc = _get_nc(T)
    in_maps = _shard_inputs(
        np.asarray(inputs["x"], dtype=np.float32),
        np.asarray(inputs["Wq"], dtype=np.float32),
        np.asarray(inputs["bq"], dtype=np.float32),
        np.asarray(inputs["Wk"], dtype=np.float32),
        np.asarray(inputs["bk"], dtype=np.float32),
        np.asarray(inputs["Wv"], dtype=np.float32),
        np.asarray(inputs["bv"], dtype=np.float32),
        np.asarray(inputs["Wp"], dtype=np.float32),
    )
    res = run_bass_kernel_spmd(
        nc, in_maps, core_ids=list(range(N_CORES)), trace=trace, **kwargs
    )
    full = _combine(res.results, np.asarray(inputs["bp"], dtype=np.float32))
    return full, res


def kernel(**inputs) -> np.ndarray:
    full, _ = _run(inputs, trace=False)
    return full
